# revision 1
# baseline (speedup 1.0000x reference)
"""Trainium2 Bass kernel for nn_ConvSPE (two depthwise convs K=201 over z).

Strategy
--------
out[t, c] = sum_j w[j, c] * z[201 + t + j, c]   (t in [0, 2048), per realization r)

Mapped to dense PE matmuls via banded-Toeplitz weight blocks: for output tile
t = 128*T + i, the contraction (i + j) splits into 3 chunks of 128 (m = 0..2):

    out[128T + i] = sum_m sum_p  W_m[p, i] * z[201 + 128(T+m) + p]
    W_m[p, i] = w[128m + p - i]   (zero outside [0, 201))

W_m is independent of T and r, so one stationary [128, 128] weight block
streams all 64 realizations x all 16 output tiles as matmul columns.

Sharding: channels across the 8 cores (64 ch = one head per core); weights and
z-slices per channel are core-private, realizations all stay on-core.

dtype: fp16 matmul inputs (11-bit mantissa -> rel err ~3e-4, full-rate PE,
half the HBM traffic of f32) accumulated in f32 PSUM; outputs stored fp16 on
device and upconverted to f32 on host (adds ~2^-11 quantization, still ~4e-4).
"""

import numpy as np
import concourse.bass as bass
import concourse.mybir as mybir
from concourse.tile import TileContext
from concourse.bass_utils import run_bass_kernel_spmd

# Problem constants (hardcoded per the task contract)
R = 64
S = 2048
K = 201
C = 512
H = 8
F = 64
PAD_LEN = 4 * K + S  # 2852
SCALE = float((R * F) ** 0.25)  # 8.0

NCORES = 8
CPC = C // NCORES      # 64 channels per core
NK = 18                # 128-element z chunks per channel: u in [201, 201 + 18*128)
NT = S // 128          # 16 output tiles
NM = 3                 # Toeplitz chunks per output tile
GROUP = 8              # channels processed per DMA group
NGROUPS = CPC // GROUP


def _round_f32r(x: np.ndarray) -> np.ndarray:
    """Round fp32 array to the float32r grid (11-bit mantissa, RNE)."""
    b = x.view(np.uint32).astype(np.uint64)
    lsb = (b >> 12) & 1
    b = (b + 0x7FF + lsb) & 0xFFFFF000
    return b.astype(np.uint32).view(np.float32)


def _split_sync_waits(nc) -> None:
    """Walrus in this container accepts at most ONE sync wait per instruction.

    Move extra on_wait entries onto same-engine InstNoOp carriers inserted
    immediately before the over-limit instruction (program order on the same
    engine preserves semantics)."""
    ctr = 0
    for f in nc.m.functions:
        for blk in f.blocks:
            new = []
            for inst in blk.instructions:
                si = inst.sync_info
                waits = list(si.on_wait) if (si is not None and si.on_wait) else []
                if len(waits) > 1:
                    for wjob in waits[:-1]:
                        nop = mybir.InstNoOp(name=f"antwaitnop{ctr}", ins=[], outs=[])
                        ctr += 1
                        nop.engine = inst.engine
                        nop.sync_info = mybir.SyncInfo(on_wait=[wjob], on_update=[])
                        new.append(nop)
                    si.on_wait = [waits[-1]]
                new.append(inst)
            blk.instructions = new


def _build_nc():
    """Build the per-core Bass program (identical on all 8 cores)."""
    nc = bass.Bass()
    f32 = mybir.dt.float32
    f16 = mybir.dt.float16

    # zt: [CPC, 128, NK*64]  layout [c][p][k*64 + r]
    zt = nc.dram_tensor("zt", [CPC, 128, NK * R], f16, kind="ExternalInput")
    # wt: [2, CPC, NM, 128, 128]  layout [conv][c][m][p][i]
    wt = nc.dram_tensor("wt", [2, CPC, NM, 128, 128], f16, kind="ExternalInput")
    # out: [2, 2048, CPC, 64]  layout [conv][t][c][r]
    out = nc.dram_tensor("out", [2, S, CPC, R], f16, kind="ExternalOutput")

    with TileContext(nc) as tc:
        with (
            tc.tile_pool(name="zpool", bufs=3) as zpool,
            tc.tile_pool(name="wpool", bufs=3) as wpool,
            tc.tile_pool(name="opool", bufs=3) as opool,
            tc.tile_pool(name="pspool", bufs=8, space="PSUM") as pspool,
        ):
            evict_ctr = 0
            for gi in range(NGROUPS):
                c0 = gi * GROUP
                # One z DMA per group: [128 p, GROUP * NK*64]
                ztile = zpool.tile([128, GROUP * NK * R], f16, tag="zt")
                src = bass.AP(
                    zt,
                    c0 * 128 * NK * R,
                    [[NK * R, 128], [128 * NK * R, GROUP], [1, NK * R]],
                )
                nc.sync.dma_start(ztile[:], src)

                wtiles = []
                for conv in range(2):
                    # One w DMA per (group, conv): [128 p, GROUP * NM * 128]
                    wtile = wpool.tile([128, GROUP * NM * 128], f16, tag="wt")
                    wsrc = bass.AP(
                        wt,
                        conv * CPC * NM * 128 * 128 + c0 * NM * 128 * 128,
                        [[128, 128], [NM * 128 * 128, GROUP], [128 * 128, NM], [1, 128]],
                    )
                    nc.sync.dma_start(wtile[:], wsrc)
                    wtiles.append(wtile)

                for conv in range(2):
                    wtile = wtiles[conv]
                    # outbuf free layout: (T, c2, r) -> contiguous 1 KiB runs in DRAM
                    outbuf = opool.tile([128, NT * GROUP * R], f16, tag="ob")
                    ob4 = outbuf[:].rearrange(
                        "p (T c r) -> p T c r", T=NT, c=GROUP, r=R
                    )
                    for c2 in range(GROUP):
                        for h in range(2):
                            ps = pspool.tile([128, 512], f32, tag="ps")
                            for m in range(NM):
                                lhsT = wtile[:, (c2 * NM + m) * 128:(c2 * NM + m + 1) * 128]
                                rhs = ztile[:, c2 * NK * R + (m + 8 * h) * R:
                                            c2 * NK * R + (m + 8 * h) * R + 512]
                                nc.tensor.matmul(
                                    ps[:], lhsT, rhs,
                                    start=(m == 0), stop=(m == NM - 1),
                                )
                            # Evict PSUM -> outbuf slice (strided dest)
                            dst = ob4[:, 8 * h:8 * h + 8, c2, :]
                            psrc = ps[:].rearrange("p (T r) -> p T r", T=8, r=R)
                            if evict_ctr % 2 == 0:
                                nc.vector.tensor_copy(dst, psrc)
                            else:
                                nc.scalar.copy(dst, psrc)
                            evict_ctr += 1
                    # One out DMA per (group, conv): contiguous (c, r) 1 KiB runs
                    odst = bass.AP(
                        out,
                        conv * S * CPC * R + c0 * R,
                        [[CPC * R, 128], [128 * CPC * R, NT], [1, GROUP * R]],
                    )
                    nc.scalar.dma_start(odst, outbuf[:])

    _split_sync_waits(nc)
    return nc


_NC_CACHE = None


def kernel(z: np.ndarray, w_q: np.ndarray, w_k: np.ndarray):
    global _NC_CACHE

    # ---- Host-side prep -------------------------------------------------
    # z slice and transpose: zt[c, p, k, r] = z[r, 201 + 128k + p, c]
    zz = np.ascontiguousarray(z[:, 201:201 + NK * 128, :]).astype(np.float16)
    zz = zz.reshape(R, NK, 128, C)                     # [r, k, p, c]
    zt = np.ascontiguousarray(zz.transpose(3, 2, 1, 0))  # [c, p, k, r]
    zt = zt.reshape(NCORES, CPC, 128, NK * R)

    # Toeplitz blocks: W[m, p, i, c] = w[128m + p - i, 0, c] / SCALE
    p = np.arange(128)[:, None]
    i = np.arange(128)[None, :]
    toep_list = []
    for w in (w_k, w_q):   # out[0] = conv with w_k (qbar), out[1] = conv with w_q (kbar)
        w = np.asarray(w, dtype=np.float32)
        blocks = np.zeros((NM, 128, 128, C), dtype=np.float32)  # fp32 build, fp16 ship
        for m in range(NM):
            J = 128 * m + p - i
            valid = (J >= 0) & (J < K)
            Jc = np.clip(J, 0, K - 1)
            blocks[m] = np.where(valid[:, :, None], w[Jc, 0, :], 0.0)
        blocks /= SCALE
        blocks = blocks.astype(np.float16)
        # -> [c, m, p, i] -> [cores, CPC, m, p, i]
        bt = np.ascontiguousarray(blocks.transpose(3, 0, 1, 2))
        toep_list.append(bt.reshape(NCORES, CPC, NM, 128, 128))
    # wt per core: [2, CPC, NM, 128, 128]
    wts = [np.ascontiguousarray(np.stack([toep_list[0][g], toep_list[1][g]]))
           for g in range(NCORES)]

    in_maps = [{"zt": np.ascontiguousarray(zt[g]), "wt": wts[g]}
               for g in range(NCORES)]

    # ---- Build + run ----------------------------------------------------
    if _NC_CACHE is None:
        _NC_CACHE = _build_nc()
    import os
    trace = bool(int(os.environ.get("KERNEL_TRACE", "0")))
    res = run_bass_kernel_spmd(
        _NC_CACHE, in_maps, core_ids=list(range(NCORES)), trace=trace,
    )
    kernel.last_result = res

    # ---- Gather ---------------------------------------------------------
    # Reference applies a RAW row-major reshape [R, S*C] -> [R, H, F, S'] then
    # transpose, so: out[conv][0, s, h, f, r] = conv[r, 256h + 4f + s//512, s % 512].
    arr = np.stack([res.results[g]["out"] for g in range(NCORES)]).astype(np.float32)
    # arr: [g, conv, t, c_local, r] -> conv_all[conv, t, c, r]
    conv_all = arr.transpose(1, 2, 0, 3, 4).reshape(2, S, C, R)
    # t = 256h + 4f + a  (row-major h, f, a); s = 512a + c
    x = conv_all.reshape(2, H, F, 4, C, R)            # [conv, h, f, a, c, r]
    x = x.transpose(0, 3, 4, 1, 2, 5).reshape(2, S, H, F, R)
    q = np.ascontiguousarray(x[0])[None]
    kk = np.ascontiguousarray(x[1])[None]
    return q, kk



# revision 3
# speedup vs baseline: 1.1702x; 1.1702x over previous
"""Trainium2 Bass kernel for nn_ConvSPE (two depthwise convs K=201 over z).

Strategy
--------
out[t, c] = sum_j w[j, c] * z[201 + t + j, c]   (t in [0, 2048), per realization r)

Mapped to dense PE matmuls via banded-Toeplitz weight blocks: for output tile
t = 128*T + i, the contraction (i + j) splits into 3 chunks of 128 (m = 0..2):

    out[128T + i] = sum_m sum_p  W_m[p, i] * z[201 + 128(T+m) + p]
    W_m[p, i] = w[128m + p - i]   (zero outside [0, 201))

The three W_m blocks per channel are slices of one 384-column "skew" tensor
    wsk[p, u] = w[256 - u + p]        (u in [0, 384), zero-padded)
with W_m[p, i] = wsk[p, (2-m)*128 + i]. Shipping wsk per-partition-contiguous
keeps every DMA run >= 512 B (full DMA rate) and makes stationary reads
contiguous.

Sharding: channels across the 8 cores (64 ch = one head per core); weights and
z-slices per channel are core-private, realizations all stay on-core.

dtype: fp16 matmul inputs accumulated in f32 PSUM; outputs stored fp16 on
device and upconverted to f32 on host (rel err ~4e-4).
"""

import numpy as np
import concourse.bass as bass
import concourse.mybir as mybir
from concourse.tile import TileContext
from concourse.bass_utils import run_bass_kernel_spmd

# Problem constants (hardcoded per the task contract)
R = 64
S = 2048
K = 201
C = 512
H = 8
F = 64
PAD_LEN = 4 * K + S  # 2852
SCALE = float((R * F) ** 0.25)  # 8.0

NCORES = 8
CPC = C // NCORES      # 64 channels per core
NK = 18                # 128-element z chunks per channel: u in [201, 201 + 18*128)
NT = S // 128          # 16 output tiles
NM = 3                 # Toeplitz chunks per output tile
GROUP = 8              # channels processed per DMA group
NGROUPS = CPC // GROUP
U = 384                # skew-tensor columns per (channel, conv)
ZSPLIT = 10            # z chunks in the first of the two per-group z DMAs


def _split_sync_waits(nc) -> None:
    """Walrus in this container accepts at most ONE sync wait per instruction.

    Move extra on_wait entries onto same-engine InstNoOp carriers inserted
    immediately before the over-limit instruction (program order on the same
    engine preserves semantics)."""
    ctr = 0
    for f in nc.m.functions:
        for blk in f.blocks:
            new = []
            for inst in blk.instructions:
                si = inst.sync_info
                waits = list(si.on_wait) if (si is not None and si.on_wait) else []
                if len(waits) > 1:
                    for wjob in waits[:-1]:
                        nop = mybir.InstNoOp(name=f"antwaitnop{ctr}", ins=[], outs=[])
                        ctr += 1
                        nop.engine = inst.engine
                        nop.sync_info = mybir.SyncInfo(on_wait=[wjob], on_update=[])
                        new.append(nop)
                    si.on_wait = [waits[-1]]
                new.append(inst)
            blk.instructions = new


def _build_nc():
    """Build the per-core Bass program (identical on all 8 cores)."""
    nc = bass.Bass()
    f32 = mybir.dt.float32
    f16 = mybir.dt.float16

    # zt: [CPC, 128, NK*64]  layout [c][p][k*64 + r]
    zt = nc.dram_tensor("zt", [CPC, 128, NK * R], f16, kind="ExternalInput")
    # wt: [128, CPC*2*U]  layout [p][(c*2 + v)*U + u]; wsk per (c, v)
    wt = nc.dram_tensor("wt", [128, CPC * 2 * U], f16, kind="ExternalInput")
    # out: [2, 2048, CPC, 64]  layout [conv][t][c][r]
    out = nc.dram_tensor("out", [2, S, CPC, R], f16, kind="ExternalOutput")

    with TileContext(nc) as tc:
        with (
            tc.tile_pool(name="zpool", bufs=3) as zpool,
            tc.tile_pool(name="wpool", bufs=3) as wpool,
            tc.tile_pool(name="opool", bufs=3) as opool,
            tc.tile_pool(name="pspool", bufs=8, space="PSUM") as pspool,
        ):
            evict_ctr = 0
            for gi in range(NGROUPS):
                c0 = gi * GROUP
                # z in two DMAs: chunks [0, ZSPLIT) unblock the h=0 matmuls,
                # chunks [ZSPLIT, NK) complete h=1.
                ztile = zpool.tile([128, GROUP * NK * R], f16, tag="zt")
                zt4 = ztile[:].rearrange(
                    "p (c k r) -> p c k r", c=GROUP, k=NK, r=R
                )
                src1 = bass.AP(
                    zt,
                    c0 * 128 * NK * R,
                    [[NK * R, 128], [128 * NK * R, GROUP], [1, ZSPLIT * R]],
                )
                nc.sync.dma_start(zt4[:, :, 0:ZSPLIT, :], src1)
                src2 = bass.AP(
                    zt,
                    c0 * 128 * NK * R + ZSPLIT * R,
                    [[NK * R, 128], [128 * NK * R, GROUP], [1, (NK - ZSPLIT) * R]],
                )
                nc.sync.dma_start(zt4[:, :, ZSPLIT:NK, :], src2)

                # One skew-weight DMA per group: [128 p, GROUP*2*U] (both convs)
                wtile = wpool.tile([128, GROUP * 2 * U], f16, tag="wt")
                wsrc = bass.AP(
                    wt, c0 * 2 * U, [[CPC * 2 * U, 128], [1, GROUP * 2 * U]]
                )
                nc.sync.dma_start(wtile[:], wsrc)

                for v in range(2):
                    # outbuf free layout: (T, c2, r) -> contiguous 1 KiB runs in DRAM
                    outbuf = opool.tile([128, NT * GROUP * R], f16, tag="ob")
                    ob4 = outbuf[:].rearrange(
                        "p (T c r) -> p T c r", T=NT, c=GROUP, r=R
                    )
                    for c2 in range(GROUP):
                        ps = [pspool.tile([128, 512], f32, tag="ps",
                                          name=f"ps{hh}")
                              for hh in range(2)]
                        ubase = (c2 * 2 + v) * U
                        for m in range(NM):
                            lhsT = wtile[:, ubase + (2 - m) * 128:
                                         ubase + (2 - m) * 128 + 128]
                            for h in range(2):
                                rhs = ztile[:, c2 * NK * R + (m + 8 * h) * R:
                                            c2 * NK * R + (m + 8 * h) * R + 512]
                                nc.tensor.matmul(
                                    ps[h][:], lhsT, rhs,
                                    start=(m == 0), stop=(m == NM - 1),
                                )
                        for h in range(2):
                            # Evict PSUM -> outbuf slice (strided dest)
                            dst = ob4[:, 8 * h:8 * h + 8, c2, :]
                            psrc = ps[h][:].rearrange("p (T r) -> p T r", T=8, r=R)
                            if evict_ctr % 2 == 0:
                                nc.vector.tensor_copy(dst, psrc)
                            else:
                                nc.scalar.copy(dst, psrc)
                            evict_ctr += 1
                    # Out DMA in two halves (T 0..7, 8..15): contiguous 1 KiB runs
                    for q in range(2):
                        odst = bass.AP(
                            out,
                            v * S * CPC * R + q * 8 * 128 * CPC * R + c0 * R,
                            [[CPC * R, 128], [128 * CPC * R, 8], [1, GROUP * R]],
                        )
                        nc.scalar.dma_start(odst, ob4[:, 8 * q:8 * q + 8, :, :])

    _split_sync_waits(nc)
    return nc


_NC_CACHE = None


def kernel(z: np.ndarray, w_q: np.ndarray, w_k: np.ndarray):
    global _NC_CACHE

    # ---- Host-side prep -------------------------------------------------
    # z slice and transpose: zt[c, p, k, r] = z[r, 201 + 128k + p, c]
    zz = np.ascontiguousarray(z[:, 201:201 + NK * 128, :]).astype(np.float16)
    zz = zz.reshape(R, NK, 128, C)                     # [r, k, p, c]
    zt = np.ascontiguousarray(zz.transpose(3, 2, 1, 0))  # [c, p, k, r]
    zt = zt.reshape(NCORES, CPC, 128, NK * R)

    # Skew tensor: wsk[p, u, c] = w[256 - u + p, 0, c] / SCALE (0 outside)
    p = np.arange(128)[:, None]
    u = np.arange(U)[None, :]
    J = 256 - u + p                     # [128, U]
    valid = (J >= 0) & (J < K)
    Jc = np.clip(J, 0, K - 1)
    wsk_list = []
    for w in (w_k, w_q):   # out[0] = conv with w_k (qbar), out[1] = conv with w_q
        w = np.asarray(w, dtype=np.float32)
        wsk = np.where(valid[:, :, None], w[Jc, 0, :], 0.0) / SCALE  # [128, U, C]
        wsk_list.append(wsk.astype(np.float16))
    # wt per core: [128, CPC, 2, U] -> [128, CPC*2*U]
    wts = []
    for g in range(NCORES):
        cs = slice(g * CPC, (g + 1) * CPC)
        wcore = np.stack([wsk_list[0][:, :, cs], wsk_list[1][:, :, cs]], axis=3)
        # wcore: [128, U, CPC, 2] -> [128, CPC, 2, U]
        wcore = np.ascontiguousarray(wcore.transpose(0, 2, 3, 1))
        wts.append(wcore.reshape(128, CPC * 2 * U))

    in_maps = [{"zt": np.ascontiguousarray(zt[g]), "wt": wts[g]}
               for g in range(NCORES)]

    # ---- Build + run ----------------------------------------------------
    if _NC_CACHE is None:
        _NC_CACHE = _build_nc()
    import os
    trace = bool(int(os.environ.get("KERNEL_TRACE", "0")))
    res = run_bass_kernel_spmd(
        _NC_CACHE, in_maps, core_ids=list(range(NCORES)), trace=trace,
    )
    kernel.last_result = res

    # ---- Gather ---------------------------------------------------------
    # Reference applies a RAW row-major reshape [R, S*C] -> [R, H, F, S'] then
    # transpose, so: out[conv][0, s, h, f, r] = conv[r, 256h + 4f + s//512, s % 512].
    arr = np.stack([res.results[g]["out"] for g in range(NCORES)]).astype(np.float32)
    # arr: [g, conv, t, c_local, r] -> conv_all[conv, t, c, r]
    conv_all = arr.transpose(1, 2, 0, 3, 4).reshape(2, S, C, R)
    # t = 256h + 4f + a  (row-major h, f, a); s = 512a + c
    x = conv_all.reshape(2, H, F, 4, C, R)            # [conv, h, f, a, c, r]
    x = x.transpose(0, 3, 4, 1, 2, 5).reshape(2, S, H, F, R)
    q = np.ascontiguousarray(x[0])[None]
    kk = np.ascontiguousarray(x[1])[None]
    return q, kk
